# revision 20
# baseline (speedup 1.0000x reference)
"""ErnieLayout self-attention on 8 Trainium2 NeuronCores (Bass/Tile). v3

Problem shapes (hardcoded): B=4, S=1024, H=768, NH=12, HD=64.
Sharding: core c -> (batch b = c//2, head-half hh = c%2, i.e. 6 heads).
Each core computes attention for its 6 heads of one batch element and
writes the [S, 384] column slice of that batch's output.

The kernel is HBM-bound: rel_pos + rel_2d_pos are 50.3 MB per core of
the ~58 MB total I/O, so the design keeps the DMA queues saturated and
sizes every engine's work under the ~160 us DMA floor (robust even when
the PE is power-throttled to 1.2 GHz, which traces show happens for most
of the kernel).

Key structure:
  * rel_pos / rel_2d_pos are uploaded HOST-TRANSPOSED per head ([k, q]
    layout, a pure layout change done while sharding).  Strips land
    contiguously; GPSIMD pre-sums rel1+rel2 in place (idle engine), and
    the DVE adds the sum straight into the transposed score PSUM with
    one RMW per [128,512] block.  No PE transposes of rel at all.
  * heads are processed in pairs (2dt, 2dt+1) whose q/k rows live in
    partitions 0-63 / 64-127 of qT/kT tile dt: the two QK score matmuls
    per (kt, qch) are emitted back-to-back and run CONCURRENTLY on the
    PE via row tiling (auto tile_position from base partitions).
  * only the X/W transposes and the d=0 Q/K projections run before the
    attention loop; the V projection and d=1,2 projections are emitted
    as fillers inside pair 0/1's kt blocks (PE slack), so pair-0
    consumption of rel strips starts ~25 us in and the 20-deep strip
    pool never backs up the DMA queue.
  * PV accumulation steps are interleaved per kt block (skip_group_
    check), so the attention tail after the last strip arrives is only
    the last block's drain + finalize.
  * scores^T layout keeps the mask as a per-partition ACT bias: masked
    keys get FLT_MIN so exp underflows to exactly 0 (no row-max needed,
    scores are O(10)).

Per-core math (identical to reference up to fp16 rounding):
  Q^T = (Wq_s @ X^T + bq)/8, K^T = Wk_s @ X^T + bk (fp16 matmuls, fp32
  PSUM), V = X @ Wv_s^T + bv stored fp16 with a ones column (col 64 ->
  softmax denominator for free).  ps[k,q] = K^T.T@Q^T (+rel12 via DVE),
  pT = exp(ps + maskbias), ctx^T[d|1, q] += V_aug[kt].T @ pT[kt],
  out[q, h*64+d] = ctx[q, d] / ctx[q, 64].
"""

import os
import sys

import numpy as np

for _p in ("/opt/trn_rl_repo",):
    if _p not in sys.path and os.path.isdir(_p):
        sys.path.append(_p)

import concourse.bass as bass
import concourse.mybir as mybir
import concourse.tile as tile
from concourse import bacc
from concourse.bass_utils import run_bass_kernel_spmd
from concourse.masks import make_identity

F32 = mybir.dt.float32
F16 = mybir.dt.float16
I32 = mybir.dt.int32
AF = mybir.ActivationFunctionType
NEG = float(np.finfo(np.float32).min)

P = 128
S = 1024
NH = 6        # heads per core
HD = 64
HIN = 768     # model dim (contraction for projections)
HOUT = NH * HD  # 384, per-core projection width
KT = S // P   # 8 key tiles
QT = S // P   # 8 query tiles
VW = HD + 1   # 65: V columns + ones column
NPAIR = NH // 2

# 'split':  GPSIMD pre-sums rel1+rel2 for head A, DVE does 2 RMWs for
#           head B (balances the two engines under the DMA pace).
# 'gpsimd': GPSIMD pre-sums everything, DVE does 1 RMW per score block.
# 'none':   DVE does 2 RMWs per score block (no pre-sum).
PRESUM = os.environ.get("K_PRESUM", "split")
PRESUM_H2 = {"gpsimd": (True, True), "split": (True, False),
             "none": (False, False)}[PRESUM]


def _build_kernel_body(tc, aps, kt_eff):
    import contextlib

    nc = tc.nc
    KTE = kt_eff
    x_ap = aps["x"]
    mask_ap = aps["mask"]
    rel1_ap = aps["rel1"]  # [NH, S(k), S(q)] -- host-transposed
    rel2_ap = aps["rel2"]
    out_ap = aps["out"]

    with contextlib.ExitStack() as ctx:
        const = ctx.enter_context(tc.tile_pool(name="const", bufs=1))

        ident32 = const.tile([P, P], F32)
        make_identity(nc, ident32)

        # long-lived tensors
        qt_pool = ctx.enter_context(tc.tile_pool(name="qT", bufs=3))
        kt_pool = ctx.enter_context(tc.tile_pool(name="kT", bufs=3))
        v_pool = ctx.enter_context(tc.tile_pool(name="v", bufs=8))
        xt_pool = ctx.enter_context(tc.tile_pool(name="xT", bufs=6))
        wt_pool = ctx.enter_context(tc.tile_pool(name="wT", bufs=6))

        qT = [qt_pool.tile([P, S], F16, tag="qT", name=f"qT{i}") for i in range(3)]
        kT = [kt_pool.tile([P, S], F16, tag="kT", name=f"kT{i}") for i in range(3)]
        v_tiles = [
            v_pool.tile([P, NH, VW], F16, tag="v", name=f"v{i}")
            for i in range(KTE)
        ]

        # rel strip pool: strip DMAs queue behind the x/W loads and then
        # stream continuously for the rest of the kernel.
        r_pool = ctx.enter_context(tc.tile_pool(name="rel", bufs=26))

        # unified PSUM pools: "bigps" carries every 1-bank use (X/W
        # transpose staging, projection groups, score tiles, finalize
        # back-transposes); "vpsum" carries the 4 ctx^T accumulators.
        bigps = ctx.enter_context(tc.tile_pool(name="bigps", bufs=4, space="PSUM"))
        vpsum = ctx.enter_context(tc.tile_pool(name="vpsum", bufs=4, space="PSUM"))

        # ---------------- phase 1a: load + cast (X, W pre-transposed) ------
        ph1 = contextlib.ExitStack()  # transient fp32 landing pools
        xload = ph1.enter_context(tc.tile_pool(name="xload", bufs=2))
        wload = ph1.enter_context(tc.tile_pool(name="wload", bufs=2))

        # X^T tiles [128(hin-chunk), 1024] fp32 -> fp16 (host-transposed)
        xT = []
        for hc in range(6):
            xt_ = xload.tile([P, S], F32, tag="x")
            nc.sync.dma_start(xt_[:], x_ap[hc * P:(hc + 1) * P, :])
            xt_t = xt_pool.tile([P, S], F16, tag="xT", name=f"xT{hc}")
            nc.scalar.copy(xt_t[:], xt_[:])
            xT.append(xt_t)

        # W^T tiles: host packs [wq^T | wk^T | wv^T] row-wise into one
        # [HIN, 3*384] tensor -> 6 big loads [128, 1152] fp32 -> fp16
        wqkv_ap = aps["wqkv"]
        wT = {}
        for hc in range(6):
            wt_ = wload.tile([P, 3, HOUT], F32, tag="wload")
            nc.sync.dma_start(wt_[:], wqkv_ap[hc * P:(hc + 1) * P, :, :])
            wt_t = wt_pool.tile(
                [P, 3, HOUT], F16, tag="wT", name=f"wT{hc}"
            )
            nc.scalar.copy(wt_t[:], wt_[:])
            for wi, wname in enumerate(("q", "k", "v")):
                wT[(wname, hc)] = wt_t[:, wi, :]

        # mask bias and projection biases (off the startup critical path)
        mask_i = const.tile([P, KTE], I32)
        nc.sync.dma_start(mask_i[:], mask_ap.rearrange("(a p) -> p a", p=P))
        maskb = const.tile([P, KTE], F32)
        nc.vector.tensor_copy(maskb[:], mask_i[:])
        nc.vector.tensor_scalar_mul(maskb[:], maskb[:], NEG)
        bias_sb = {}
        for wname in ("q", "k"):
            bt = const.tile([P, 3], F32, tag=f"b{wname}")
            nc.sync.dma_start(
                bt[:], aps[f"b{wname}"].rearrange("(a p) -> p a", p=P)
            )
            if wname == "q":
                nc.vector.tensor_scalar_mul(bt[:], bt[:], 0.125)
            bias_sb[wname] = bt
        bv_bc = const.tile([P, NH, HD], F32)
        nc.sync.dma_start(
            bv_bc[:],
            aps["bv"].rearrange("(h d) -> h d", d=HD)[None].to_broadcast(
                (P, NH, HD)
            ),
        )

        def emit_qk_proj(wname, d, tch):
            dest = qT if wname == "q" else kT
            scale = 0.125 if wname == "q" else 1.0
            pp = bigps.tile([P, 512], F32, tag="ps")
            for hc in range(6):
                nc.tensor.matmul(
                    pp[:],
                    wT[(wname, hc)][:, d * P:(d + 1) * P],
                    xT[hc][:, tch * 512:(tch + 1) * 512],
                    start=(hc == 0),
                    stop=(hc == 5),
                )
            nc.scalar.activation(
                dest[d][:, tch * 512:(tch + 1) * 512],
                pp[:],
                AF.Identity,
                bias=bias_sb[wname][:, d:d + 1],
                scale=scale,
            )

        def emit_v_proj(t):
            pv = bigps.tile([P, 512], F32, tag="ps", name="pv")[:, :HOUT]
            for hc in range(6):
                nc.tensor.matmul(
                    pv[:],
                    xT[hc][:, t * P:(t + 1) * P],
                    wT[("v", hc)][:],
                    start=(hc == 0),
                    stop=(hc == 5),
                )
            nc.vector.memset(v_tiles[t][:, :, HD:HD + 1], 1.0)
            nc.vector.tensor_add(
                v_tiles[t][:, :, 0:HD],
                pv[:].rearrange("p (h d) -> p h d", d=HD),
                bv_bc[:],
            )

        # d=0 projections (pair 0's heads) + V tile 0 up front; the rest
        # are fillers emitted inside pair 0/1's kt blocks.
        for wname in ("q", "k"):
            for tch in range(2):
                emit_qk_proj(wname, 0, tch)
        emit_v_proj(0)

        # fillers[dt][kt] -> list of closures to emit at that block
        fillers = [[[] for _ in range(KTE)] for _ in range(NPAIR)]
        for t in range(1, KTE):  # V tile t needed by pair-0 block kt=t
            fillers[0][t - 1].append(lambda t=t: emit_v_proj(t))
        for i, (wname, tch) in enumerate(
            (w, t) for w in ("q", "k") for t in range(2)
        ):
            blk = min(i, KTE - 1)
            fillers[0][blk].append(
                lambda w=wname, t=tch: emit_qk_proj(w, 1, t)
            )
            fillers[1][blk].append(
                lambda w=wname, t=tch: emit_qk_proj(w, 2, t)
            )

        # transient load/cast pools are only read by the phase-1a
        # transposes; free their SBUF for the phase-2 pools
        ph1.close()

        # ---------------- phase 2: attention per head pair -----------------
        out_pool = ctx.enter_context(tc.tile_pool(name="outst", bufs=8))
        out_stage = [
            out_pool.tile([P, HOUT], F16, tag="outst", name=f"outst{i}")
            for i in range(8)
        ]
        pt_pool = ctx.enter_context(tc.tile_pool(name="pT", bufs=8))
        fin_pool = ctx.enter_context(tc.tile_pool(name="fin", bufs=4))
        ctt_pool = ctx.enter_context(tc.tile_pool(name="ctt", bufs=4))

        def emit_fin_copy(fin, ctxT_sb, h2s=(0, 1)):
            """ACT-copy the previous pair's ctx^T accumulators out of PSUM
            (releases the vpsum banks for this pair's PV groups)."""
            dt, ctxT_ps = fin
            for h2 in h2s:
                for qch in range(2):
                    t_ = ctt_pool.tile(
                        [VW, 512], F32, tag="ctxT_sb", name=f"ctT{dt}_{h2}_{qch}"
                    )
                    nc.scalar.copy(t_[:], ctxT_ps[(h2, qch)][:])
                    ctxT_sb[(h2, qch)] = t_
            return ctxT_sb

        def emit_fin_rest(fin, ctxT_sb, h2s, emit_out_dma):
            """Back-transpose ctx^T per head, divide by the denominator,
            write out_stage (and the output DMAs for the last pair)."""
            dt, _ = fin
            for h2 in h2s:
                h = 2 * dt + h2
                ctx_ps = [
                    bigps.tile([P, 512], F32, tag="ps", name=f"ctx{h}_{i}")
                    for i in range(2)
                ]
                for qt in range(QT):
                    cp = ctx_ps[qt // 4]
                    sl = (qt % 4) * VW
                    nc.tensor.transpose(
                        cp[:, sl:sl + VW],
                        ctxT_sb[(h2, qt // 4)][:, (qt % 4) * P:(qt % 4 + 1) * P],
                        ident32[:VW, :VW],
                    )
                rc4 = []
                for i in range(2):
                    rc = fin_pool.tile([P, 4], F32, tag="recip")
                    denoms = ctx_ps[i][:, 0:4 * VW].rearrange(
                        "p (a b) -> p a b", b=VW
                    )[:, :, HD]
                    nc.vector.reciprocal(rc[:], denoms)
                    rc4.append(rc)
                for qt in range(QT):
                    cp = ctx_ps[qt // 4]
                    sl = (qt % 4) * VW
                    nc.scalar.activation(
                        out_stage[qt][:, h * HD:(h + 1) * HD],
                        cp[:, sl:sl + HD],
                        AF.Identity,
                        scale=rc4[qt // 4][:, qt % 4:qt % 4 + 1],
                    )
                    if emit_out_dma and h2 == 1:
                        nc.sync.dma_start(
                            out_ap[qt * P:(qt + 1) * P, :], out_stage[qt][:]
                        )

        pending_fin = None
        for dt in range(NPAIR):
            # rel strips for both heads: [k=128, q=1024] fp32, kt-major,
            # heads interleaved to match consumption order.
            r1 = [[None] * KTE for _ in range(2)]
            r2 = [[None] * KTE for _ in range(2)]
            for kt in range(KTE):
                eng = nc.scalar if (dt == 0 and kt < 4) else nc.sync
                for h2 in range(2):
                    h = 2 * dt + h2
                    t1 = r_pool.tile([P, S], F32, tag="rel", name=f"r1_{h}_{kt}")
                    eng.dma_start(t1[:], rel1_ap[h][kt * P:(kt + 1) * P, :])
                    r1[h2][kt] = t1
                    t2 = r_pool.tile([P, S], F32, tag="rel", name=f"r2_{h}_{kt}")
                    eng.dma_start(t2[:], rel2_ap[h][kt * P:(kt + 1) * P, :])
                    r2[h2][kt] = t2

            if pending_fin is not None:
                fin_sb = {}
                emit_fin_copy(pending_fin, fin_sb)

            ctxT_ps = {}
            for qch in range(2):
                for h2 in range(2):
                    ctxT_ps[(h2, qch)] = vpsum.tile(
                        [VW, 512], F32, tag="ctxT", name=f"ctxT{dt}_{h2}_{qch}"
                    )

            # kt blocks: strips fully consumed within their block; PV
            # accumulation steps interleaved so the tail after the last
            # strip is only one block's drain.
            fin_a = max(0, KTE - 4)
            fin_b = max(fin_a + 1, KTE - 2)
            for kt in range(KTE):
                pT_kt = [
                    pt_pool.tile([P, S], F16, tag="pT", name=f"pT{dt}_{h2}_{kt}")
                    for h2 in range(2)
                ]
                presum_h2 = PRESUM_H2
                for h2 in range(2):
                    if presum_h2[h2]:
                        nc.gpsimd.tensor_add(
                            r1[h2][kt][:], r1[h2][kt][:], r2[h2][kt][:]
                        )
                ps4 = {}
                for qch in range(2):
                    qsl = slice(qch * 512, (qch + 1) * 512)
                    for h2 in range(2):
                        d0 = h2 * HD
                        ps = bigps.tile([P, 512], F32, tag="ps")
                        # back-to-back K=64 matmuls at base partitions 0/64
                        # get distinct row-group tile_positions -> run
                        # concurrently on the PE array
                        nc.tensor.matmul(
                            ps[:],
                            kT[dt][d0:d0 + HD, kt * P:(kt + 1) * P],
                            qT[dt][d0:d0 + HD, qsl],
                            start=True,
                            stop=True,
                        )
                        ps4[(qch, h2)] = ps
                for f in fillers[dt][kt]:
                    f()
                for qch in range(2):
                    qsl = slice(qch * 512, (qch + 1) * 512)
                    for h2 in range(2):
                        ps = ps4[(qch, h2)]
                        nc.vector.tensor_add(ps[:], ps[:], r1[h2][kt][:, qsl])
                        if not presum_h2[h2]:
                            nc.vector.tensor_add(
                                ps[:], ps[:], r2[h2][kt][:, qsl]
                            )
                        nc.scalar.activation(
                            pT_kt[h2][:, qsl],
                            ps[:],
                            AF.Exp,
                            bias=maskb[:, kt:kt + 1],
                            scale=1.0,
                        )
                # PV steps for this kt (both heads x both q-chunks)
                for qch in range(2):
                    qsl = slice(qch * 512, (qch + 1) * 512)
                    for h2 in range(2):
                        h = 2 * dt + h2
                        nc.tensor.matmul(
                            ctxT_ps[(h2, qch)][:],
                            v_tiles[kt][:, h, :],
                            pT_kt[h2][:, qsl],
                            start=(kt == 0),
                            stop=(kt == KTE - 1),
                            skip_group_check=True,
                        )
                # previous pair's finalize, spread mid-pair so it never
                # lands in the post-DMA tail
                if pending_fin is not None and kt in (fin_a, fin_b):
                    emit_fin_rest(pending_fin, fin_sb,
                                  (0 if kt == fin_a else 1,),
                                  emit_out_dma=False)

            pending_fin = (dt, ctxT_ps)

        # last pair: copies on the DVE (ACT is draining exps), then both
        # heads' scales interleaved per q-tile with its output DMA right
        # behind, so the store stream pipelines with the scale stream.
        dt_l, ctxT_l = pending_fin
        fin_sb = {}
        for h2 in range(2):
            for qch in range(2):
                t_ = ctt_pool.tile(
                    [VW, 512], F32, tag="ctxT_sb", name=f"ctTL_{h2}_{qch}"
                )
                nc.vector.tensor_copy(t_[:], ctxT_l[(h2, qch)][:])
                fin_sb[(h2, qch)] = t_
        ctx_ps_l = {}
        rc4_l = {}
        for h2 in range(2):
            cps = [
                bigps.tile([P, 512], F32, tag="ps", name=f"lctx{h2}_{i}")
                for i in range(2)
            ]
            for qt in range(QT):
                cp = cps[qt // 4]
                sl = (qt % 4) * VW
                nc.tensor.transpose(
                    cp[:, sl:sl + VW],
                    fin_sb[(h2, qt // 4)][:, (qt % 4) * P:(qt % 4 + 1) * P],
                    ident32[:VW, :VW],
                )
            for i in range(2):
                rc = fin_pool.tile([P, 4], F32, tag="recip")
                denoms = cps[i][:, 0:4 * VW].rearrange(
                    "p (a b) -> p a b", b=VW
                )[:, :, HD]
                nc.vector.reciprocal(rc[:], denoms)
                rc4_l[(h2, i)] = rc
            ctx_ps_l[h2] = cps
        for qt in range(QT):
            for h2 in range(2):
                h = 2 * dt_l + h2
                cp = ctx_ps_l[h2][qt // 4]
                sl = (qt % 4) * VW
                nc.scalar.activation(
                    out_stage[qt][:, h * HD:(h + 1) * HD],
                    cp[:, sl:sl + HD],
                    AF.Identity,
                    scale=rc4_l[(h2, qt // 4)][:, qt % 4:qt % 4 + 1],
                )
            nc.sync.dma_start(
                out_ap[qt * P:(qt + 1) * P, :], out_stage[qt][:]
            )


def build_program(kt_eff=8):
    """Build and compile the per-core Bass program. Returns nc."""
    nc = bacc.Bacc(
        "TRN2",
        target_bir_lowering=False,
        debug=False,
        num_devices=8,
    )
    aps = {
        "x": nc.dram_tensor("x", [HIN, S], F32, kind="ExternalInput").ap(),
        "mask": nc.dram_tensor("mask", [kt_eff * P], I32, kind="ExternalInput").ap(),
        "rel1": nc.dram_tensor("rel1", [NH, kt_eff * P, S], F32, kind="ExternalInput").ap(),
        "rel2": nc.dram_tensor("rel2", [NH, kt_eff * P, S], F32, kind="ExternalInput").ap(),
        "wqkv": nc.dram_tensor(
            "wqkv", [HIN, 3, HOUT], F32, kind="ExternalInput"
        ).ap(),
        "bq": nc.dram_tensor("bq", [HOUT], F32, kind="ExternalInput").ap(),
        "bk": nc.dram_tensor("bk", [HOUT], F32, kind="ExternalInput").ap(),
        "bv": nc.dram_tensor("bv", [HOUT], F32, kind="ExternalInput").ap(),
        "out": nc.dram_tensor("out", [S, HOUT], F16, kind="ExternalOutput").ap(),
    }
    with tile.TileContext(nc) as tc:
        _build_kernel_body(tc, aps, kt_eff)
    nc.compile()
    return nc


def make_perms(inputs):
    """Per batch: a sequence permutation putting unmasked keys first, and
    the uniform key-tile count kt_eff = max_b ceil(#unmasked / 128).

    Masked keys (attention_mask == 1) get score FLT_MIN in the reference,
    so their rel values and V rows cannot affect the output: after the
    permutation the kernel only streams/computes the first kt_eff*128
    keys.  All 1024 queries are kept."""
    am = np.asarray(inputs["attention_mask"]).astype(np.int32)[:, 0, 0, :]
    perms = [np.argsort(am[b], kind="stable") for b in range(4)]
    kt_eff = max(int(-(-int((am[b] == 0).sum()) // P)) for b in range(4))
    kt_eff = max(1, min(KT, kt_eff))
    return perms, kt_eff


def make_in_maps(inputs, perms, kt_eff):
    """Slice full inputs into the 8 per-core input maps.

    All uploads are permuted by the batch's sequence permutation (pure
    layout): x and rel transposed, rel sliced to the kept key rows."""
    hs = np.ascontiguousarray(np.asarray(inputs["hidden_states"], np.float32))
    am = np.asarray(inputs["attention_mask"]).astype(np.int32)
    rel1 = np.asarray(inputs["rel_pos"], np.float32)
    rel2 = np.asarray(inputs["rel_2d_pos"], np.float32)
    ws = {k: np.asarray(inputs["W" + k[-1]], np.float32) for k in ("wq", "wk", "wv")}
    bs = {k: np.asarray(inputs["b" + k[-1]], np.float32) for k in ("bq", "bk", "bv")}

    nk = kt_eff * P
    in_maps = []
    for c in range(8):
        b, hh = divmod(c, 2)
        perm = perms[b]
        kperm = perm[:nk]
        hsl = slice(hh * NH, (hh + 1) * NH)
        csl = slice(hh * HOUT, (hh + 1) * HOUT)

        def relT(r):
            # [6, k', q'] = r[perm[q'], perm[k']]^T, kept key rows only
            rt = r[b, hsl].transpose(0, 2, 1)  # [6, k, q] view
            return np.ascontiguousarray(rt[:, kperm, :][:, :, perm])

        m = {
            "x": np.ascontiguousarray(hs[b].T[:, perm]),
            "mask": np.ascontiguousarray(am[b, 0, 0][kperm]),
            "rel1": relT(rel1),
            "rel2": relT(rel2),
        }
        m["wqkv"] = np.ascontiguousarray(
            np.stack([ws[k][csl].T for k in ("wq", "wk", "wv")], axis=1)
        )
        for k in ("bq", "bk", "bv"):
            m[k] = np.ascontiguousarray(bs[k][csl])
        in_maps.append(m)
    return in_maps


def gather_output(results, perms):
    out = np.empty((4, S, HIN), np.float32)
    for c in range(8):
        b, hh = divmod(c, 2)
        out[b, perms[b], hh * HOUT:(hh + 1) * HOUT] = results[c]["out"]
    return out


_NC_CACHE = {}


def kernel(**inputs):
    perms, kt_eff = make_perms(inputs)
    if kt_eff not in _NC_CACHE:
        _NC_CACHE[kt_eff] = build_program(kt_eff)
    nc = _NC_CACHE[kt_eff]
    in_maps = make_in_maps(inputs, perms, kt_eff)
    res = run_bass_kernel_spmd(nc, in_maps, list(range(8)))
    return gather_output(res.results, perms)
